# revision 4
# baseline (speedup 1.0000x reference)
"""AdaptiveCrossEntropyLoss on 8 TRN2 NeuronCores (Bass/Tile).

Vocab-parallel sharding: each core owns 1/8 of every cluster's rows
(2048+2048+1024+1024 = 6144 rows of W). Each core computes its shard's
logits for all 4096 tokens with float32r matmuls and reduces them with an
online (flash-style) softmax into per-(token, cluster) stats
(max, sumexp, argmax). One AllToAll exchanges stats so core k owns tokens
[512k, 512k+512); each core then merges the 8 shards' stats, adds the exact
target logit (host-gathered W[t] rows, fp32 dot on-device) and the router
log-softmax term, and writes per-token NLL + closest. The host stitches the
8 output slices and computes the scalar loss / used count.
"""

import numpy as np
from contextlib import ExitStack

from concourse import bass, bacc, tile
import concourse.mybir as mybir
from concourse.bass_utils import run_bass_kernel_spmd

# ---- problem constants (hardcoded; kernel.py must be self-contained) ----
VOCAB_SIZES = [16384, 16384, 8192, 8192]
CUTOFFS = [0, 16384, 32768, 40960, 49152]
V_TOTAL = 49152
DIM = 1024
N_TOK = 4096          # 2 * 2048
B, S = 2, 2048
NCORES = 8
IGNORE_INDEX = -100

KB = DIM // 128       # 8 k-blocks
VLOC = V_TOTAL // NCORES          # 6144 local vocab columns
SEG = [v // NCORES for v in VOCAB_SIZES]     # [2048, 2048, 1024, 1024]
SEG_LO = [0, 2048, 4096, 5120]
CH = 1024             # chunk width for softmax reduction
# chunk -> (cluster, is_first_chunk_of_cluster)
CHUNKS = [(0, True), (0, False), (1, True), (1, False), (2, True), (3, True)]
NCHUNK = len(CHUNKS)

N_GRP = 2             # token groups (x re-residency); weights streamed per group
TILES_PER_GRP = N_TOK // 128 // N_GRP   # 16
TOK_PER_CORE = N_TOK // NCORES          # 512
TILES_PER_CORE = TOK_PER_CORE // 128    # 4

F32 = mybir.dt.float32
F32R = mybir.dt.float32r
U32 = mybir.dt.uint32
EXP = mybir.ActivationFunctionType.Exp
LN = mybir.ActivationFunctionType.Ln
BIG = float(2 ** 23)

_CACHE = {}


def build():
    nc = bacc.Bacc("TRN2", target_bir_lowering=False, debug=False,
                   num_devices=NCORES)

    # ---- per-core inputs ----
    xT = nc.dram_tensor("xT", [DIM, N_TOK], F32R, kind="ExternalInput").ap()
    wT = nc.dram_tensor("wT", [DIM, VLOC], F32R, kind="ExternalInput").ap()
    xTo = nc.dram_tensor("xTo", [DIM, TOK_PER_CORE], F32, kind="ExternalInput").ap()
    rwT = nc.dram_tensor("rwT", [DIM, 4], F32, kind="ExternalInput").ap()
    x_tok = nc.dram_tensor("x_tok", [TOK_PER_CORE, DIM], F32, kind="ExternalInput").ap()
    wt_tok = nc.dram_tensor("wt_tok", [TOK_PER_CORE, DIM], F32, kind="ExternalInput").ap()
    onehot = nc.dram_tensor("onehot", [TOK_PER_CORE, 4], F32, kind="ExternalInput").ap()
    idx_base = nc.dram_tensor("idx_base", [128, 4], F32, kind="ExternalInput").ap()

    out_nll = nc.dram_tensor("out_nll", [TOK_PER_CORE, 1], F32, kind="ExternalOutput").ap()
    out_cls = nc.dram_tensor("out_cls", [TOK_PER_CORE, 1], F32, kind="ExternalOutput").ap()

    with tile.TileContext(nc) as tc, ExitStack() as ctx:
        xp = ctx.enter_context(tc.tile_pool(name="xp", bufs=1))
        wp = ctx.enter_context(tc.tile_pool(name="wp", bufs=3))
        sp = ctx.enter_context(tc.tile_pool(name="sp", bufs=4))
        stp = ctx.enter_context(tc.tile_pool(name="stp", bufs=TILES_PER_GRP))
        cp = ctx.enter_context(tc.tile_pool(name="cp", bufs=2))
        psum = ctx.enter_context(tc.tile_pool(name="ps", bufs=3, space="PSUM"))
        psr = ctx.enter_context(tc.tile_pool(name="psr", bufs=2, space="PSUM"))
        dram = ctx.enter_context(tc.tile_pool(name="dram", bufs=1, space="DRAM"))

        bounce_in = dram.tile([N_TOK, 12], F32, name="bounce_in")
        bounce_out = dram.tile([N_TOK, 12], F32, name="bounce_out")

        # small per-core constants
        base_sb = cp.tile([128, 4], F32, name="base_sb", bufs=1)
        nc.sync.dma_start(base_sb[:], idx_base)

        # ---------------- main vocab-parallel loop ----------------
        xT_r = xT.rearrange("(kb p) m -> p kb m", p=128)
        wT_r = wT.rearrange("(kb p) v -> p kb v", p=128)

        for grp in range(N_GRP):
            gt0 = grp * TILES_PER_GRP        # first tile of group
            x_sb = xp.tile([128, KB, TILES_PER_GRP * 128], F32R, name="x_sb")
            nc.sync.dma_start(
                x_sb[:],
                xT_r[:, :, gt0 * 128 : (gt0 + TILES_PER_GRP) * 128],
            )

            # per-tile online state for this group
            m_run = [stp.tile([128, 4], F32, name="m_run") for _ in range(TILES_PER_GRP)]
            s_run = [stp.tile([128, 4], F32, name="s_run") for _ in range(TILES_PER_GRP)]
            vidx = [stp.tile([128, 4], F32, name="vidx") for _ in range(TILES_PER_GRP)]

            for ci, (cl, first) in enumerate(CHUNKS):
                # stream this chunk's weights as two 512-wide halves
                w_h = []
                for h in range(2):
                    wt = wp.tile([128, KB, 512], F32R, name="w_h")
                    nc.sync.dma_start(
                        wt[:],
                        wT_r[:, :, ci * CH + h * 512 : ci * CH + (h + 1) * 512],
                    )
                    w_h.append(wt)

                for t in range(TILES_PER_GRP):
                    ps = psum.tile([128, CH], F32, name="ps")
                    for k in range(KB):
                        for h in range(2):
                            nc.tensor.matmul(
                                ps[:, h * 512 : (h + 1) * 512],
                                lhsT=x_sb[:, k, t * 128 : (t + 1) * 128],
                                rhs=w_h[h][:, k, :],
                                start=(k == 0),
                                stop=(k == KB - 1),
                            )

                    inm = sp.tile([128, 8], F32, name="inm")
                    nc.vector.memset(inm[:], -1e30)
                    nc.vector.reduce_max(inm[:, 0:1], ps[:], axis=mybir.AxisListType.X)
                    idx8 = sp.tile([128, 8], U32, name="idx8")
                    nc.vector.max_index(idx8[:], inm[:], ps[:])
                    idxf = sp.tile([128, 1], F32, name="idxf")
                    nc.vector.tensor_copy(idxf[:], idx8[:, 0:1])

                    nm = sp.tile([128, 1], F32, name="nm")
                    scr = sp.tile([128, CH], F32, name="scr", bufs=2)
                    mc = inm[:, 0:1]
                    if first:
                        nc.vector.tensor_scalar_mul(nm[:], mc, -1.0)
                        nc.vector.tensor_copy(m_run[t][:, cl : cl + 1], mc)
                        nc.scalar.activation(
                            scr[:], ps[:], EXP, bias=nm[:],
                            accum_out=s_run[t][:, cl : cl + 1],
                        )
                        nc.vector.tensor_scalar_add(
                            vidx[t][:, cl : cl + 1], idxf[:], float(ci * CH)
                        )
                    else:
                        mo = m_run[t][:, cl : cl + 1]
                        mnew = sp.tile([128, 1], F32, name="mnew")
                        nc.vector.tensor_max(mnew[:], mo, mc)
                        nc.vector.tensor_scalar_mul(nm[:], mnew[:], -1.0)
                        upd = sp.tile([128, 1], F32, name="upd")
                        nc.vector.tensor_tensor(
                            upd[:], mnew[:], mo, op=mybir.AluOpType.is_gt
                        )
                        r = sp.tile([128, 1], F32, name="r")
                        nc.scalar.activation(r[:], mo, EXP, bias=nm[:])
                        sc = sp.tile([128, 1], F32, name="sc")
                        nc.scalar.activation(
                            scr[:], ps[:], EXP, bias=nm[:], accum_out=sc[:]
                        )
                        nc.vector.scalar_tensor_tensor(
                            s_run[t][:, cl : cl + 1],
                            s_run[t][:, cl : cl + 1], r[:], sc[:],
                            op0=mybir.AluOpType.mult, op1=mybir.AluOpType.add,
                        )
                        gi = sp.tile([128, 1], F32, name="gi")
                        nc.vector.tensor_scalar_add(gi[:], idxf[:], float(ci * CH))
                        d = sp.tile([128, 1], F32, name="d")
                        nc.vector.tensor_sub(d[:], gi[:], vidx[t][:, cl : cl + 1])
                        nc.vector.scalar_tensor_tensor(
                            vidx[t][:, cl : cl + 1],
                            d[:], upd[:], vidx[t][:, cl : cl + 1],
                            op0=mybir.AluOpType.mult, op1=mybir.AluOpType.add,
                        )
                        nc.vector.tensor_copy(mo, mnew[:])

            # pack + ship stats for this group
            for t in range(TILES_PER_GRP):
                stats = sp.tile([128, 12], F32, name="stats")
                nc.vector.tensor_copy(stats[:, 0:4], m_run[t][:])
                nc.vector.tensor_copy(stats[:, 4:8], s_run[t][:])
                nc.vector.tensor_add(stats[:, 8:12], vidx[t][:], base_sb[:])
                r0 = (gt0 + t) * 128
                nc.sync.dma_start(bounce_in[r0 : r0 + 128, :], stats[:])

        # ---------------- exchange ----------------
        nc.gpsimd.collective_compute(
            "AllToAll",
            mybir.AluOpType.bypass,
            replica_groups=[list(range(NCORES))],
            ins=[bounce_in.opt()],
            outs=[bounce_out.opt()],
        )

        # ---------------- owned-token work (router + target dot) ----------
        rw_sb = cp.tile([128, KB, 4], F32, name="rw_sb", bufs=1)
        nc.sync.dma_start(rw_sb[:], rwT.rearrange("(kb p) c -> p kb c", p=128))
        xo_sb = cp.tile([128, KB, TOK_PER_CORE], F32, name="xo_sb", bufs=1)
        nc.sync.dma_start(xo_sb[:], xTo.rearrange("(kb p) m -> p kb m", p=128))

        comb_r = bounce_out[:].rearrange("(c t) s -> t c s", c=NCORES)

        for t in range(TILES_PER_CORE):
            tsl = slice(t * 128, (t + 1) * 128)

            # router logits for owned tokens
            psr_t = psr.tile([128, 4], F32, name="psr_t")
            for k in range(KB):
                nc.tensor.matmul(
                    psr_t[:],
                    lhsT=xo_sb[:, k, tsl],
                    rhs=rw_sb[:, k, :],
                    start=(k == 0),
                    stop=(k == KB - 1),
                )

            # target logit: rowwise dot of x and W[target]
            xt_sb = cp.tile([128, DIM], F32, name="xt_sb")
            nc.sync.dma_start(xt_sb[:], x_tok[tsl, :])
            wt_sb = cp.tile([128, DIM], F32, name="wt_sb")
            nc.sync.dma_start(wt_sb[:], wt_tok[tsl, :])
            oh_sb = cp.tile([128, 4], F32, name="oh_sb")
            nc.sync.dma_start(oh_sb[:], onehot[tsl, :])

            prod = cp.tile([128, DIM], F32, name="prod", bufs=1)
            tl = cp.tile([128, 1], F32, name="tl")
            nc.vector.scalar_tensor_tensor(
                prod[:], xt_sb[:], 1.0, wt_sb[:],
                op0=mybir.AluOpType.mult, op1=mybir.AluOpType.mult,
                accum_out=tl[:],
            )

            # merge the 8 cores' stats
            comb = cp.tile([128, NCORES, 12], F32, name="comb")
            nc.sync.dma_start(comb[:], comb_r[tsl, :, :])

            M4 = cp.tile([128, 4], F32, name="M4")
            S5 = cp.tile([128, 5], F32, name="S5")
            IDX4 = cp.tile([128, 4], F32, name="IDX4")
            for c in range(4):
                mview = comb[:, :, c]
                nc.vector.reduce_max(
                    M4[:, c : c + 1], mview, axis=mybir.AxisListType.X
                )
                mb = M4[:, c : c + 1].broadcast_to([128, NCORES])
                delta = cp.tile([128, NCORES], F32, name="delta")
                nc.vector.tensor_sub(delta[:], mview, mb)
                e8 = cp.tile([128, NCORES], F32, name="e8")
                nc.scalar.activation(e8[:], delta[:], EXP)
                pr8 = cp.tile([128, NCORES], F32, name="pr8")
                nc.vector.tensor_mul(pr8[:], e8[:], comb[:, :, 4 + c])
                nc.vector.reduce_sum(
                    S5[:, c : c + 1], pr8[:], axis=mybir.AxisListType.X
                )
                eq8 = cp.tile([128, NCORES], F32, name="eq8")
                nc.vector.tensor_tensor(
                    eq8[:], mview, mb, op=mybir.AluOpType.is_equal
                )
                i1 = cp.tile([128, NCORES], F32, name="i1")
                nc.vector.tensor_scalar_add(i1[:], comb[:, :, 8 + c], -BIG)
                i2 = cp.tile([128, NCORES], F32, name="i2")
                nc.vector.tensor_mul(i2[:], i1[:], eq8[:])
                i3 = cp.tile([128, NCORES], F32, name="i3")
                nc.vector.tensor_scalar_add(i3[:], i2[:], BIG)
                nc.vector.tensor_reduce(
                    IDX4[:, c : c + 1], i3[:],
                    op=mybir.AluOpType.min, axis=mybir.AxisListType.X,
                )

            # router: m, sumexp into S5[:, 4]
            rm = cp.tile([128, 1], F32, name="rm")
            nc.vector.reduce_max(rm[:], psr_t[:], axis=mybir.AxisListType.X)
            nrm = cp.tile([128, 1], F32, name="nrm")
            nc.vector.tensor_scalar_mul(nrm[:], rm[:], -1.0)
            rexp = cp.tile([128, 4], F32, name="rexp")
            nc.scalar.activation(
                rexp[:], psr_t[:], EXP, bias=nrm[:], accum_out=S5[:, 4:5]
            )
            rdot = cp.tile([128, 1], F32, name="rdot")
            rpr = cp.tile([128, 4], F32, name="rpr")
            nc.vector.scalar_tensor_tensor(
                rpr[:], psr_t[:], 1.0, oh_sb[:],
                op0=mybir.AluOpType.mult, op1=mybir.AluOpType.mult,
                accum_out=rdot[:],
            )

            # single Ln over [cluster sums | router sum]
            LN5 = cp.tile([128, 5], F32, name="LN5")
            nc.scalar.activation(LN5[:], S5[:], LN)

            # lse_c = M_c + ln(S_c)  (clusters only)
            lse4 = cp.tile([128, 4], F32, name="lse4")
            nc.vector.tensor_add(lse4[:], M4[:], LN5[:, 0:4])

            # select target cluster's lse and closest
            lsel = cp.tile([128, 1], F32, name="lsel")
            sel1 = cp.tile([128, 4], F32, name="sel1")
            nc.vector.scalar_tensor_tensor(
                sel1[:], lse4[:], 1.0, oh_sb[:],
                op0=mybir.AluOpType.mult, op1=mybir.AluOpType.mult,
                accum_out=lsel[:],
            )
            csel = cp.tile([128, 1], F32, name="csel")
            sel2 = cp.tile([128, 4], F32, name="sel2")
            nc.vector.scalar_tensor_tensor(
                sel2[:], IDX4[:], 1.0, oh_sb[:],
                op0=mybir.AluOpType.mult, op1=mybir.AluOpType.mult,
                accum_out=csel[:],
            )

            # router log-prob of target cluster:
            #   rsel = rdot - rm - ln(rs)
            # nll = lsel - tl - rsel
            rb = cp.tile([128, 1], F32, name="rb")
            nc.vector.tensor_add(rb[:], rm[:], LN5[:, 4:5])
            n1 = cp.tile([128, 1], F32, name="n1")
            nc.vector.tensor_sub(n1[:], lsel[:], tl[:])
            n2 = cp.tile([128, 1], F32, name="n2")
            nc.vector.tensor_sub(n2[:], n1[:], rdot[:])
            n3 = cp.tile([128, 1], F32, name="n3")
            nc.vector.tensor_add(n3[:], n2[:], rb[:])

            nc.sync.dma_start(out_nll[tsl, :], n3[:])
            nc.sync.dma_start(out_cls[tsl, :], csel[:])

    nc.compile()
    return nc


def _host_prep(input, target, weight, router_weight):
    x = np.ascontiguousarray(np.asarray(input, dtype=np.float32)).reshape(N_TOK, DIM)
    t = np.ascontiguousarray(np.asarray(target, dtype=np.int32)).reshape(N_TOK)
    w = np.ascontiguousarray(np.asarray(weight, dtype=np.float32))
    rw = np.ascontiguousarray(np.asarray(router_weight, dtype=np.float32))

    xT = np.ascontiguousarray(x.T)                      # [DIM, N_TOK]
    rwT = np.ascontiguousarray(rw.T)                    # [DIM, 4]

    # cluster id and within-cluster position for every token
    cl = np.searchsorted(np.asarray(CUTOFFS[1:]), t, side="right").astype(np.int32)
    # rel position in cluster
    rel = t - np.asarray(CUTOFFS, dtype=np.int32)[cl]

    wt_all = w[np.clip(t, 0, V_TOTAL - 1)]              # [N_TOK, DIM]
    onehot_all = np.zeros((N_TOK, 4), np.float32)
    onehot_all[np.arange(N_TOK), cl] = 1.0

    in_maps = []
    for k in range(NCORES):
        rows = np.concatenate(
            [
                np.arange(CUTOFFS[c] + SEG[c] * k, CUTOFFS[c] + SEG[c] * (k + 1))
                for c in range(4)
            ]
        )
        wTk = np.ascontiguousarray(w[rows].T)           # [DIM, VLOC]
        tok = slice(k * TOK_PER_CORE, (k + 1) * TOK_PER_CORE)
        base = np.tile(
            np.array(
                [[CUTOFFS[c] + SEG[c] * k - SEG_LO[c] for c in range(4)]],
                np.float32,
            ),
            (128, 1),
        )
        in_maps.append(
            {
                "xT": xT,
                "wT": wTk,
                "xTo": np.ascontiguousarray(xT[:, tok]),
                "rwT": rwT,
                "x_tok": np.ascontiguousarray(x[tok]),
                "wt_tok": np.ascontiguousarray(wt_all[tok]),
                "onehot": np.ascontiguousarray(onehot_all[tok]),
                "idx_base": base,
            }
        )
    return in_maps, t


def kernel(input, target, weight, router_weight, _trace=False):
    if "nc" not in _CACHE:
        _CACHE["nc"] = build()
    nc = _CACHE["nc"]

    in_maps, t = _host_prep(input, target, weight, router_weight)
    res = run_bass_kernel_spmd(
        nc, in_maps, core_ids=list(range(NCORES)), trace=_trace
    )
    _CACHE["last_result"] = res

    nll = np.concatenate(
        [res.results[k]["out_nll"][:, 0] for k in range(NCORES)]
    ).astype(np.float32)
    closest = np.concatenate(
        [res.results[k]["out_cls"][:, 0] for k in range(NCORES)]
    )

    used = np.int32(((t >= 0) & (t < V_TOTAL)).sum())
    loss = np.float32(nll.sum() / max(int(used), 1))
    return (
        loss,
        used,
        nll.reshape(B, S),
        np.rint(closest).astype(np.int32).reshape(B, S),
    )
